# revision 1
# baseline (speedup 1.0000x reference)
"""Trainium2 Bass kernel for nn_BiLSTM_2491081031886.

Single-layer unidirectional LSTM (B=2048, T=256, F=H=128) + Linear([T*H]->1).
Data-parallel over 8 NeuronCores: each core owns a 256-row batch shard and
runs the full sequential scan locally; weights are replicated.

Per-core dataflow (v2, decoupled waves), all layouts [hidden, batch]:
  - x is pre-transposed and cast to bf16 on the host ([F, T, BS] per shard),
    so chunked plain DMA lands it directly in the matmul-ready layout.
  - Gate pre-activations accumulate in PSUM as 4 blocks [i|f|o|2g] x 256 cols
    per step, in three bank-aligned rotating buffers (cols 0/1024/2048):
      bias (K=4 matmul vs a block-indicator, bf16)
      + W_ih^T.T @ xT_t (bf16, N=256)
      + W_hh2^T.T @ h_half (bf16, N=128 per wave)
  - Three independent batch waves (cols 0:86/86:171/171:256) run the
    recurrence with no cross-wave data dependency; their serial chains
    interleave on the engines, hiding the per-step matmul->sigmoid->DVE
    dependency-ring latency.  Per wave and step:
      sg    = Sigmoid(blocks)                   # one packed ACT op, bf16 out
      t2h   = (sg2g - 0.5) * sgi                # DVE stt, bf16
      u     = sgf * cd_prev                     # DVE tt (cd fp32 SBUF)
      cd    = 4*t2h + u                         # DVE stt, fp32
      scd   = Sigmoid(cd)                       # small ACT op, bf16
      h_half= (scd - 0.5) * sgo                 # DVE stt, bf16
    with tanh realized via sigmoid (weights pre-scaled x2 on the g-chunk,
    doubled cell state cd = 2c) and the h/2 factor absorbed into 2x on
    W_hh and w_lin.
  - Output head: acc[1,wave] += (2*w_lin_t) as lhsT against h_half (bf16),
    accumulated in PSUM over all steps; +b_lin on host.  The three wlin
    accumulators live in separate PSUM banks (4/5/6): a start=True matmul
    resets has_written for its whole bank, so they must not share one.
"""

import numpy as np
import ml_dtypes

import concourse.bacc as bacc
import concourse.bass as bass
import concourse.mybir as mybir
from concourse import tile
from concourse.bass_utils import run_bass_kernel_spmd

F32 = mybir.dt.float32
BF16 = mybir.dt.bfloat16
AF = mybir.ActivationFunctionType
OP = mybir.AluOpType

B, T_FULL, F = 2048, 256, 128
H = F
NCORES = 8
BS = B // NCORES  # 256 batch rows per core
WAVES = (("A", 0, 86), ("B", 86, 85), ("C", 171, 85))
W2 = 128
TC = 8            # timesteps per x-ingest chunk

# PSUM column layout (fp32 words per partition, 4096 total = 8 banks x 512)
PS_BUF = (0, 1024)     # two step buffers, 2 banks each (banks 0-3)
BLK = 256              # block width: [i|f|o|2g] each 256 cols (A:0-127 B:128+)
WLIN = {"A": 2048, "B": 2560, "C": 3072}  # banks 4/5/6, one per wave


def build(T=T_FULL, ablate=(), period_ns=0.0):
    """ablate: timing-only experiment knobs (results become wrong):
    'wlin' drop output-head matmuls; 'xin' drop x ingest (static xT);
    'dve' drop cell-math DVE ops; 'bias' drop bias matmuls; 'act' drop
    sigma ops; 'rec' drop recurrent matmuls.
    period_ns > 0 paces the scan at that many ns per step via virtual
    not-before times (tile_wait_until), phase-shifting the two waves."""
    ablate = set(ablate)
    assert T % 2 == 0
    nc = bacc.Bacc("TRN2", target_bir_lowering=False, debug=False,
                   num_devices=NCORES)

    xt_d = nc.dram_tensor("xt", [F, T, BS], BF16, kind="ExternalInput")
    whh_d = nc.dram_tensor("whh", [H, 4 * H], BF16, kind="ExternalInput")
    wih_d = nc.dram_tensor("wih", [F, 4 * H], BF16, kind="ExternalInput")
    b4_d = nc.dram_tensor("b4", [4, H], BF16, kind="ExternalInput")
    e4_d = nc.dram_tensor("e4", [4, 1024], BF16, kind="ExternalInput")
    wl_d = nc.dram_tensor("wl", [H, T], BF16, kind="ExternalInput")
    out_d = nc.dram_tensor("out", [BS], F32, kind="ExternalOutput")

    n_chunks = (T + TC - 1) // TC
    no_dve = "dve" in ablate
    no_act = "act" in ablate

    with tile.TileContext(nc) as tc:
        with (
            tc.tile_pool(name="const", bufs=1) as constp,
            tc.tile_pool(name="xT", bufs=3) as xtp,
            tc.tile_pool(name="sig", bufs=3) as sigp,
            tc.tile_pool(name="hh", bufs=3) as hhp,
            tc.tile_pool(name="cd", bufs=3) as cdp,
            tc.tile_pool(name="tmp", bufs=3) as tmpp,
            tc.tile_pool(name="psum", bufs=1, space=bass.MemorySpace.PSUM) as psp,
        ):
            # ---- constants ----
            whh = constp.tile([H, 4 * H], BF16)
            wih = constp.tile([F, 4 * H], BF16)
            b4 = constp.tile([4, H], BF16)
            e4 = constp.tile([4, 1024], BF16)
            wl = constp.tile([H, T], BF16)
            nc.sync.dma_start(whh[:], whh_d.ap())
            nc.sync.dma_start(wih[:], wih_d.ap())
            nc.sync.dma_start(b4[:], b4_d.ap())
            nc.sync.dma_start(e4[:], e4_d.ap())
            nc.sync.dma_start(wl[:], wl_d.ap())

            ps = psp.tile([128, 4096], F32)

            # ---- x ingest: host pre-transposed [F, T, BS] bf16, plain
            # chunked DMA straight into the matmul-ready layout ----
            xtap = xt_d.ap()
            xchunks = []
            if "xin" in ablate:
                x0 = constp.tile([F, TC, BS], BF16)
                nc.sync.dma_start(x0[:], xtap[:, 0:TC, :])
                xchunks = [x0] * n_chunks
            else:
                for ch in range(n_chunks):
                    t0 = ch * TC
                    tc_n = min(TC, T - t0)
                    xc = xtp.tile([F, TC, BS], BF16)
                    nc.sync.dma_start(xc[:, 0:tc_n, :], xtap[:, t0:t0 + tc_n, :])
                    xchunks.append(xc)

            def make_xt(t):
                return xchunks[t // TC][:, t % TC, :]

            # ---- state tiles ----
            wv_w = {w: n for w, _, n in WAVES}
            cd_prev = {}
            for w, _, wn in WAVES:
                t0c = cdp.tile([H, wn], F32, tag=f"cd{w}")
                nc.vector.memset(t0c[:], 0.0)
                cd_prev[w] = t0c
            if no_dve:
                hh_s = {}
                for w, _, wn in WAVES:
                    hh_s[w] = hhp.tile([H, wn], BF16, tag=f"hh{w}")
                    nc.vector.memset(hh_s[w][:], 0.01)
            if no_act:
                s_s = {}
                for w, _, wn in WAVES:
                    s_s[w] = sigp.tile([128, 4, wn], BF16, tag=f"s{w}")
                    nc.vector.memset(s_s[w][:], 0.5)
                scd_s = tmpp.tile([H, 86], BF16, tag="scds")
                nc.vector.memset(scd_s[:], 0.5)

            hh_prev = {w: None for w, _, _ in WAVES}
            s_last = {w: None for w, _, _ in WAVES}
            wv_co = {w: c for w, c, _ in WAVES}

            cd_cur = {}

            def wave_p1(t, w):
                """Recurrent matmuls + the packed gates sigma."""
                base = PS_BUF[t % 2]
                co, wn = wv_co[w], wv_w[w]
                if hh_prev[w] is not None and "rec" not in ablate:
                    for c in range(4):
                        nc.tensor.matmul(
                            ps[:, base + c * BLK + co:base + c * BLK + co + wn],
                            whh[:, c * H:(c + 1) * H], hh_prev[w][:],
                            start=False, stop=False, skip_group_check=True)
                blocks = ps[:, base:base + 1024].rearrange(
                    "p (c n) -> p c n", c=4)
                if no_act:
                    s = s_s[w]
                else:
                    s = sigp.tile([128, 4, wn], BF16, tag=f"s{w}")
                    nc.scalar.activation(s[:], blocks[:, :, co:co + wn],
                                         AF.Sigmoid)
                s_last[w] = s

            def wave_p2(t, w):
                """Cell-state DVE chain: t2h, u, cd."""
                if no_dve:
                    return
                s = s_last[w]
                wn = wv_w[w]
                t2h = tmpp.tile([H, wn], BF16, tag=f"t2h{w}")
                nc.vector.scalar_tensor_tensor(
                    t2h[:], s[:, 3, :], -0.5, s[:, 0, :], OP.add, OP.mult)
                u = tmpp.tile([H, wn], F32, tag=f"u{w}")
                nc.vector.tensor_tensor(u[:], s[:, 1, :], cd_prev[w][:],
                                        OP.mult)
                cd = cdp.tile([H, wn], F32, tag=f"cd{w}")
                nc.vector.scalar_tensor_tensor(
                    cd[:], t2h[:], 4.0, u[:], OP.mult, OP.add)
                cd_prev[w] = cd
                cd_cur[w] = cd

            def wave_p3(t, w):
                """sigma(cd), h_half, and the output-head matmul."""
                if no_dve:
                    hh_prev[w] = hh_s[w]
                    return
                s = s_last[w]
                wn = wv_w[w]
                if no_act:
                    scd = scd_s
                else:
                    scd = tmpp.tile([H, wn], BF16, tag=f"scd{w}")
                    nc.scalar.activation(scd[:], cd_cur[w][:], AF.Sigmoid)
                hh = hhp.tile([H, wn], BF16, tag=f"hh{w}")
                nc.vector.scalar_tensor_tensor(
                    hh[:], scd[:], -0.5, s[:, 2, :], OP.add, OP.mult)
                hh_prev[w] = hh
                # output head (wlin_B never uses start=True: shared bank 6)
                if "wlin" not in ablate:
                    acc = WLIN[w]
                    nc.tensor.matmul(
                        ps[0:1, acc:acc + wn], wl[:, t:t + 1], hh[:],
                        start=(t == 0), stop=(t == T - 1),
                        skip_group_check=True)

            # ---- main scan: wave B is emitted one step behind wave A so the
            # scheduler phase-shifts the two independent serial chains ----
            first = "bias" in ablate
            from contextlib import nullcontext

            def paced(t, w, ph):
                """not-before window: wave A at t*P, wave B half a period
                later; phases p1/p2/p3 at -0.1/+0.3/+0.55 of a period."""
                if not period_ns:
                    return nullcontext()
                base_t = (t + (0.5 if w == "B" else 0.0)) * period_ns
                off = {1: -0.1, 2: 0.3, 3: 0.55}[ph] * period_ns
                return tc.tile_wait_until(max(0.0, base_t + off) * 1e-6)

            for slot in range(T + 1):
                if slot >= 1:
                    with paced(slot - 1, "B", 1):
                        wave_p1(slot - 1, "B")
                        wave_p1(slot - 1, "C")
                if slot < T:
                    base = PS_BUF[slot % 2]
                    xt = make_xt(slot)
                    with paced(slot, "A", 1):
                        if "bias" not in ablate:
                            nc.tensor.matmul(
                                ps[:, base:base + 512], b4[:], e4[:, 0:512],
                                start=True, stop=False, skip_group_check=True)
                            nc.tensor.matmul(
                                ps[:, base + 512:base + 1024], b4[:],
                                e4[:, 512:1024],
                                start=True, stop=False, skip_group_check=True)
                        for c in range(4):
                            nc.tensor.matmul(
                                ps[:, base + c * BLK:base + (c + 1) * BLK],
                                wih[:, c * H:(c + 1) * H], xt,
                                start=first, stop=False, skip_group_check=True)
                        wave_p1(slot, "A")
                if slot >= 1:
                    with paced(slot - 1, "B", 2):
                        wave_p2(slot - 1, "B")
                        wave_p2(slot - 1, "C")
                if slot < T:
                    with paced(slot, "A", 2):
                        wave_p2(slot, "A")
                if slot >= 1:
                    with paced(slot - 1, "B", 3):
                        wave_p3(slot - 1, "B")
                        wave_p3(slot - 1, "C")
                if slot < T:
                    with paced(slot, "A", 3):
                        wave_p3(slot, "A")

            # output
            outsb = constp.tile([1, 2 * W2], F32)
            for w, co, wn in WAVES:
                nc.vector.tensor_copy(outsb[0:1, co:co + wn],
                                      ps[0:1, WLIN[w]:WLIN[w] + wn])
            nc.sync.dma_start(out_d.ap().rearrange("(a b) -> a b", a=1),
                              outsb[:])

    nc.compile()
    return nc


_CACHE = {}


def _get_nc(T=T_FULL):
    if T not in _CACHE:
        _CACHE[T] = build(T)
    return _CACHE[T]


def prep_weights(w_ih, w_hh, b_ih, b_hh, w_lin, T=T_FULL):
    """Host-side weight prep. Chunk order [i, f, o, g]; g-chunk pre-scaled x2
    (sigmoid(2g) trick); W_hh and w_lin pre-scaled x2 (h_half absorption)."""
    perm = np.r_[0:H, H:2 * H, 3 * H:4 * H, 2 * H:3 * H]
    gs = np.ones((4 * H, 1), np.float32)
    gs[3 * H:] = 2.0
    bf = ml_dtypes.bfloat16
    whh = np.ascontiguousarray((w_hh[perm] * gs * 2.0).T.astype(bf))
    wih = np.ascontiguousarray((w_ih[perm] * gs).T.astype(bf))
    b4 = ((b_ih + b_hh)[perm] * gs[:, 0]).reshape(4, H).astype(bf)
    e4 = np.zeros((4, 1024), bf)
    for c in range(4):
        e4[c, c * 256:(c + 1) * 256] = 1.0
    wl = np.ascontiguousarray((2.0 * w_lin.reshape(T, H)).T.astype(bf))
    return whh, wih, b4, e4, wl


def prep_x(x):
    """Shard + host-transpose x to [F, T, BS] bf16 per core (the layout the
    xg matmuls consume, so no on-chip transpose is needed)."""
    xb = x.astype(ml_dtypes.bfloat16)
    return [np.ascontiguousarray(xb[c * BS:(c + 1) * BS].transpose(2, 1, 0))
            for c in range(NCORES)]


def kernel(x, w_ih, w_hh, b_ih, b_hh, w_lin, b_lin):
    x = np.asarray(x, np.float32)
    T = x.shape[1]
    nc = _get_nc(T)
    whh, wih, b4, e4, wl = prep_weights(
        np.asarray(w_ih, np.float32), np.asarray(w_hh, np.float32),
        np.asarray(b_ih, np.float32), np.asarray(b_hh, np.float32),
        np.asarray(w_lin, np.float32), T)
    xts = prep_x(x)
    in_maps = []
    for c in range(NCORES):
        in_maps.append({
            "xt": xts[c],
            "whh": whh, "wih": wih, "b4": b4, "e4": e4, "wl": wl,
        })
    res = run_bass_kernel_spmd(nc, in_maps, core_ids=list(range(NCORES)))
    out = np.concatenate([r["out"] for r in res.results])
    return (out + np.float32(b_lin[0])).astype(np.float32)

